# revision 33
# baseline (speedup 1.0000x reference)
"""Trainium2 Bass kernel for nn_CosSimSpatTempConvNet.

Math (reference):
  merged[f,c,k] = conv_w[f,k] * spat_w[f,c]                  (rank-1 kernel)
  conved[b,f,t] = sum_{c,k} merged[f,c,k] * x[b,c,t+k]       (valid conv, Tout=T-K+1)
  norm_w[f]    = ||conv_w[f]|| * ||spat_w[f]||
  norm_in[b,t] = sqrt(sum_{c,k} x[b,c,t+k]^2)
  cos[b,f,t]   = conved * 64 / (norm_w[f] * norm_in[b,t])
  out[b]       = sum_f (mean_t |cos[b,f,t]| * weight[f] + bias[f])

Device strategy (8 cores, data-parallel over batch, 8 b per core):
  * Conv as fp8e4 DoubleRow TensorE matmuls: contraction 256 per
    instruction = 128 partitions (k2 in {0,1} x c) x 2 k-planes
    (j in {0,1}; k = 2*kq + k2 + 32*j).  16 PSUM-accumulated matmuls per
    512-wide time tile (vs 32 for fp32r), 0.5 cycles/output-row.
    Stationaries wdr[(k2,c), kq, j, f] = conv_w[f,2kq+k2+32j]*spat_w[f,c]
    host-precomputed, scaled by SW=128, clipped to +-240 (TRN e4m3).
  * Weights use the SwInterleave layout (host pre-interleaved pairs);
    matmuls run at ~97% of the fp8 TensorE roofline (~216ns per
    512-wide DR matmul incl. its LDWEIGHTS).
  * Moving operand xf8[(k2,c), j, t] = 16*x[c, t+k2+32j] in fp8, built
    by ScalarE/DVE casts + one SBUF shift-DMA.  Batches are processed
    in PAIRS ([128, T] x-load, one Square, [2, 512] c-sum matmuls);
    the odd batch keeps its planes on partitions 64-127 and uses the
    row-rolled stationary wdr_o, so no cross-partition engine ops.
  * norm_in: x^2 (ScalarE), sum over c via ones-stationary matmul,
    then per-pair chunked layout [32 chunks, 256+64 halo] so the
    sliding-window-64 doubling shifts + reciprocal run on short rows.
  * epilogue per (b, t-tile): 1/norm row staged to DRAM and DMA-
    broadcast across 128 partitions (partition_broadcast AP, no PE);
    |conved| via ScalarE Abs; fused multiply+row-sum on DVE
    (accum_out), all in bf16.
  * software pipeline: pair p+1's load/casts/squares/norms are emitted
    between pair p's convs; next-pair c-sum matmuls interleave between
    conv waves to keep the PE stream dense.  A post-pass rewrites
    back-to-back duplicate InstLdweights into NOPs.
  * finish: S[f,b] sums -> one matmul with stationary
    g[f] = 64*weight/(4033*norm_w*SW*SX) contracting over f, + sum(bias).
"""

import contextlib
import ctypes
import sys
import types

import numpy as np

import concourse.bass as bass
import concourse.mybir as mybir
import concourse.tile as tile
from concourse.bass_utils import run_bass_kernel_spmd
from concourse.vector_clock import ScopedClock

F32 = mybir.dt.float32
F32R = mybir.dt.float32r
BF16 = mybir.dt.bfloat16
F8 = mybir.dt.float8e4

B, CIN, T = 64, 64, 4096
F, K = 128, 64
TOUT = T - K + 1          # 4033
NCORES = 8
BLOC = B // NCORES        # 8 batches per core
KQ = 16                   # DoubleRow quad groups: k = 2*kq + k2 + 32*j
TS = 512                  # moving-operand tile (one fp32 PSUM bank)
NTILES = (TOUT + TS - 1) // TS      # 8 (last tile 449)
SCALE = 64.0              # sqrt(CIN*K)
SW = 128.0                # fp8 weight scale
SX = 16.0                 # fp8 x scale
NCH = 16                  # norm chunks per batch
CW = 256                  # chunk output width
CPAD = 320                # chunk width incl. 64-halo
WAVE = 3                  # conv tiles accumulating concurrently (PSUM banks)

AF = mybir.ActivationFunctionType
ALU = mybir.AluOpType
DR = mybir.MatmulPerfMode.DoubleRowSwInterleave


# ---------------------------------------------------------------------------
# Container fixups: walrus here rejects >1 sem-wait on a Drain; TileContext's
# tail drain carries one wait per logical processor.  Chunk into single-wait
# drains.  Also recreate the (absent) antenv.axon_hooks NTFF profile hook so
# trace=True works when a test harness wants timings.
# ---------------------------------------------------------------------------

def _patched_drain_and_barrier(self, tick_clock, wait_clock):
    nc = self.nc
    drain_inst = nc.sync.drain()
    wait_clock.add_sem_waits(
        drain_inst.ins, ScopedClock({None: tick_clock.global_clock})
    )
    si = drain_inst.ins.sync_info
    waits = list(si.on_wait or []) if si else []
    if len(waits) > 1:
        si.on_wait = waits[:1]
        for w in waits[1:]:
            d2 = nc.sync.drain()
            si2 = d2.ins.sync_info
            if si2 is None:
                d2.ins.sync_info = mybir.SyncInfo(on_wait=[w], on_update=[])
            else:
                si2.on_wait = [w]
    nc.all_engine_barrier()
    assert self.sems is not None
    popped = nc._tile_sem_poison_stack.pop()
    assert popped is self._sem_poison
    nc.clear_and_free_semaphores(list(self.sems.allocated().values()))
    nc.all_engine_barrier()


def _install_ntff_hook():
    if "antenv.axon_hooks" in sys.modules:
        return
    try:
        lib = ctypes.CDLL("/opt/axon/libaxon_pjrt.so")
    except OSError:
        return
    if not hasattr(lib, "axon_start_nrt_profile"):
        return
    lib.axon_start_nrt_profile.argtypes = [
        ctypes.POINTER(ctypes.c_int64),
        ctypes.c_size_t,
    ]
    lib.axon_start_nrt_profile.restype = ctypes.c_int64
    lib.axon_stop_nrt_profile.argtypes = [ctypes.c_char_p]
    lib.axon_stop_nrt_profile.restype = ctypes.c_int64

    @contextlib.contextmanager
    def _hook(output_dir, device_ids):
        import jax

        jax.devices()
        if device_ids:
            ids = (ctypes.c_int64 * len(device_ids))(*device_ids)
            rc = lib.axon_start_nrt_profile(ids, len(device_ids))
        else:
            rc = lib.axon_start_nrt_profile(None, 0)
        if rc != 0:
            raise RuntimeError(f"axon_start_nrt_profile rc={rc}")
        try:
            yield
        finally:
            n = lib.axon_stop_nrt_profile(str(output_dir).encode())
            print(f"profile: {n} ntff file(s) in {output_dir}", file=sys.stderr)

    mod = types.ModuleType("antenv.axon_hooks")
    mod.get_axon_ntff_profile_hook = lambda: _hook
    mod.set_axon_ntff_profile_hook = lambda h: None
    import antenv

    antenv.axon_hooks = mod
    sys.modules["antenv.axon_hooks"] = mod


_ORIG_COMMIT = tile.TileContext._commit_instruction


def _commit_split_waits(self, inst, lazy_reg_writes=True):
    """walrus here allows only one sem-wait per instruction; move extras
    onto same-engine NOPs committed immediately before the instruction."""
    si = getattr(inst, "sync_info", None)
    if (
        si is not None
        and si.on_wait
        and len(si.on_wait) > 1
        and inst.engine != mybir.EngineType.Unassigned
    ):
        waits = list(si.on_wait)
        si.on_wait = waits[:1]
        for i, w in enumerate(waits[1:]):
            nop = mybir.InstNoOp(
                name=f"{inst.name}-wsplit{i}", ins=[], outs=[]
            )
            nop.engine = inst.engine
            nop.sync_info = mybir.SyncInfo(on_wait=[w], on_update=[])
            _ORIG_COMMIT(self, nop, lazy_reg_writes=False)
    return _ORIG_COMMIT(self, inst, lazy_reg_writes)


def install_fixups():
    tile.TileContext._drain_and_barrier = _patched_drain_and_barrier
    tile.TileContext._commit_instruction = _commit_split_waits
    _install_ntff_hook()


def dedupe_ldweights(nc: bass.Bass) -> int:
    """Replace back-to-back InstLdweights that reload the identical
    stationary with same-sync NOPs.  walrus emits one LDWEIGHTS per
    non-f32 matmul; in a kq-outer wave the followers reload the same
    weights, costing ~120ns of PE each for nothing.  Weights persist in
    the array across matmuls, so a NOP carrying the original sync_info
    is semantically equivalent."""
    n = 0
    for fn in nc.m.functions:
        for bb in fn.blocks:
            last_key = None
            insts = bb.instructions
            for i, inst in enumerate(insts):
                if isinstance(inst, mybir.InstLdweights):
                    key = repr(inst.ins)
                    if key == last_key:
                        nop = mybir.InstNoOp(
                            name=f"{inst.name}-wdup", ins=[], outs=[]
                        )
                        nop.engine = inst.engine
                        nop.sync_info = inst.sync_info
                        insts[i] = nop
                        n += 1
                    else:
                        last_key = key
                elif isinstance(inst, mybir.InstMatmult):
                    if inst.ldweights is not False:
                        last_key = None
                elif getattr(inst, "engine", None) == getattr(
                    mybir.EngineType, "PE", None
                ):
                    last_key = None
    return n


# ---------------------------------------------------------------------------
# Device program (identical on all 8 cores; inputs differ per core)
# ---------------------------------------------------------------------------

def build_program() -> bass.Bass:
    install_fixups()
    nc = bass.Bass()

    xs_in = nc.dram_tensor("xs", [BLOC, CIN, T], F32, kind="ExternalInput")
    wdr_in = nc.dram_tensor("wdr", [128, KQ, 2 * F], F8, kind="ExternalInput")
    wdro_in = nc.dram_tensor("wdro", [128, KQ, 2 * F], F8, kind="ExternalInput")
    g_in = nc.dram_tensor("g", [F, 1], F32, kind="ExternalInput")
    bsum_in = nc.dram_tensor("bsum", [1, 1], F32, kind="ExternalInput")
    ones2_in = nc.dram_tensor("ones2", [128, 2], BF16, kind="ExternalInput")
    out_d = nc.dram_tensor("out", [1, BLOC], F32, kind="ExternalOutput")
    rdram = nc.dram_tensor("rdram", [BLOC, NCH, CW], BF16, kind="Internal")

    PS = bass.MemorySpace.PSUM

    with tile.TileContext(nc) as tc:
        with (
            tc.tile_pool(name="const", bufs=1) as constp,
            tc.tile_pool(name="xtp", bufs=2) as xtp,
            tc.tile_pool(name="sqp", bufs=2) as sqp,
            tc.tile_pool(name="xf8p", bufs=4) as xf8p,
            tc.tile_pool(name="rowp", bufs=2) as rowp,
            tc.tile_pool(name="slidep", bufs=3) as slidep,
            tc.tile_pool(name="rcp", bufs=2) as rcp,
            tc.tile_pool(name="pbsp", bufs=3) as pbsp,
            tc.tile_pool(name="scrp", bufs=3) as scrp,
            tc.tile_pool(name="accp", bufs=2) as accp,
            tc.tile_pool(name="pconv", bufs=2 * WAVE, space=PS) as pconv,
            tc.tile_pool(name="psq", bufs=2, space=PS) as psq,
        ):
            # constants
            wdr = constp.tile([128, KQ, 2 * F], F8)
            nc.sync.dma_start(wdr[:], wdr_in[:])
            wdr_o = constp.tile([128, KQ, 2 * F], F8)
            nc.sync.dma_start(wdr_o[:], wdro_in[:])
            gsb = constp.tile([F, 1], F32)
            nc.sync.dma_start(gsb[:], g_in[:])
            bsumsb = constp.tile([1, 1], F32)
            nc.sync.dma_start(bsumsb[:], bsum_in[:])
            ones2 = constp.tile([128, 2], BF16)
            nc.sync.dma_start(ones2[:], ones2_in[:])

            S = constp.tile([F, BLOC], F32)        # per-(f,b) |cos| sums

            def load_x(p):
                """DMA x for batches (2p, 2p+1)."""
                xt2 = xtp.tile([128, T], F32, tag="xt", name=f"xt{p}")
                nc.sync.dma_start(xt2[:], xs_in[2 * p:2 * p + 2])
                return xt2

            def square_pair(p, xt2):
                """One Square for the pair + norm staging row."""
                xsq2 = sqp.tile([128, T], BF16, tag="xsq", name=f"xsq{p}")
                nc.scalar.activation(xsq2[:], xt2[:], AF.Square)
                srow2 = rowp.tile(
                    [2, NCH + 1, CW], BF16, tag="srow", name=f"srow{p}"
                )
                nc.vector.memset(srow2[0:2, NCH, :], 1.0)
                return {"xsq": xsq2, "srow": srow2}

            def prep_half(p, xt2, h):
                """fp8 moving operand for batch 2p+h via ScalarE/DVE casts.
                The odd batch writes its (k2=0) planes on partitions 64-127
                (same lanes as its x rows); conv then uses the row-rolled
                stationary wdr_o."""
                xf8 = xf8p.tile(
                    [128, 2, T], F8, tag="xf8", name=f"xf8_{2 * p + h}"
                )
                lo, hi = (0, 64) if h == 0 else (64, 128)
                src = xt2[lo:hi, :]
                nc.vector.memset(xf8[:, 0:2, T - 64:T], 0.0)
                nc.scalar.activation(
                    xf8[lo:hi, 0, 0:T], src[:, 0:T], AF.Copy, scale=SX
                )
                nc.vector.tensor_scalar_mul(
                    xf8[lo:hi, 1, 0:T - 32], src[:, 32:T], SX
                )
                # shifted (k2=1) planes land on the other 64 partitions
                olo = 64 - lo
                nc.sync.dma_start(
                    xf8[olo:olo + 64, 0:2, 0:T - 1],
                    xf8[lo:hi, 0:2, 1:T],
                )
                return xf8

            def sq_mms(st, tslist):
                """Pair c-sum matmuls (interleavable between conv waves):
                ones2 columns pick upper/lower 64 partitions -> [2, TS]."""
                for ts in tslist:
                    pq = psq.tile([2, TS], F32, name="pq", tag="pq")
                    nc.tensor.matmul(
                        pq[:],
                        ones2[:],
                        st["xsq"][:, ts * TS:(ts + 1) * TS],
                    )
                    nc.vector.tensor_copy(
                        st["srow"][0:2, 2 * ts:2 * ts + 2, :], pq[:]
                    )

            def norm_pair(p, st):
                """Sliding-window-64 sums + 1/sqrt for a batch pair;
                results to rdram[2p], rdram[2p+1]."""
                b0 = 2 * p
                srow2 = st["srow"]
                sqc = slidep.tile([2 * NCH, CPAD], BF16, tag="slide",
                                  name=f"sqc{p}")
                for h in range(2):
                    r0 = NCH * h
                    nc.sync.dma_start(
                        sqc[r0:r0 + NCH, 0:CW],
                        srow2[h:h + 1, 0:NCH, :],
                    )
                    nc.sync.dma_start(
                        sqc[r0:r0 + NCH, CW:CPAD],
                        srow2[h:h + 1, 1:NCH + 1, 0:64],
                    )
                cur = sqc
                width = CPAD
                for sh in (1, 2, 4, 8, 16, 32):
                    width -= sh
                    nxt = slidep.tile([2 * NCH, CPAD], BF16, tag="slide",
                                      name=f"sl{p}_{sh}")
                    nc.vector.tensor_tensor(
                        nxt[:, 0:width], cur[:, 0:width],
                        cur[:, sh:sh + width], op=ALU.add,
                    )
                    cur = nxt
                assert width == CW + 1
                rec = rowp.tile([2 * NCH, CPAD], F32, tag="rec",
                                name=f"rec{p}")
                nc.vector.reciprocal(rec[:, 0:CW], cur[:, 0:CW])
                rc = rcp.tile([2 * NCH, CW], BF16, tag="rc", name=f"rc{p}")
                nc.scalar.activation(rc[:], rec[:, 0:CW], AF.Sqrt)
                nc.sync.dma_start(rdram[b0], rc[0:NCH, :])
                nc.sync.dma_start(rdram[b0 + 1], rc[NCH:2 * NCH, :])

            def conv_batch(b, xf8, wtile, inter_st, sq_span):
                """Conv waves + cosine epilogue; sq matmuls of the *next*
                pair (inter_st) are interleaved between waves to keep the
                PE stream dense."""
                lo, hi = sq_span
                acc = accp.tile([F, NTILES], F32, name=f"acc{b}")
                sq_chunks = [(0, 3), (3, 6), (6, 8)]
                for wi, w0 in enumerate(range(0, NTILES, WAVE)):
                    wtiles = list(range(w0, min(w0 + WAVE, NTILES)))
                    pcs = {}
                    for ts in wtiles:
                        pcs[ts] = pconv.tile(
                            [F, TS], F32, name=f"pc_{b}_{ts}", tag="pc"
                        )
                    # kq-outer: one stationary serves len(wtiles) matmuls;
                    # followers skip the redundant LDWEIGHTS
                    for kq in range(KQ):
                        for i, ts in enumerate(wtiles):
                            t0 = ts * TS
                            nt = min(TS, TOUT - t0)
                            nt_mm = nt + (nt & 1)
                            bi = nc.tensor.matmul(
                                pcs[ts][:, 0:nt_mm],
                                wtile[:, kq, :],
                                xf8[:, 0:2, t0 + 2 * kq: t0 + 2 * kq + nt_mm],
                                start=(kq == 0),
                                stop=(kq == KQ - 1),
                                perf_mode=DR,
                            )
                            if i > 0:
                                bi.ins.ldweights = False
                    if inter_st is not None:
                        lo, hi = sq_chunks[wi]
                        sq_mms(inter_st, range(lo, hi))
                    for ts in wtiles:
                        t0 = ts * TS
                        nt = min(TS, TOUT - t0)
                        # 1/norm row, broadcast across partitions by DMA
                        # from the DRAM copy (not PE)
                        pbs = pbsp.tile(
                            [128, TS], BF16, tag="pbs", name=f"pbs{b}_{ts}"
                        )
                        nc.sync.dma_start(
                            pbs[:],
                            rdram[b, 2 * ts:2 * ts + 2, 0:CW]
                            .partition_broadcast(128),
                        )
                        # DVE may read only one PSUM operand: take |conv| on
                        # ScalarE (PSUM->SBUF), then multiply+row-sum on DVE.
                        scr = scrp.tile(
                            [F, TS], BF16, tag="scr", name=f"scr{b}_{ts}"
                        )
                        nc.scalar.activation(
                            scr[:, 0:nt], pcs[ts][:, 0:nt], AF.Abs
                        )
                        nc.vector.scalar_tensor_tensor(
                            scr[:, 0:nt],
                            scr[:, 0:nt],
                            1.0,
                            pbs[:, 0:nt],
                            op0=ALU.mult,
                            op1=ALU.mult,
                            accum_out=acc[:, ts:ts + 1],
                        )
                nc.vector.reduce_sum(
                    S[:, b:b + 1], acc[:], axis=mybir.AxisListType.X
                )

            # software pipeline over batch pairs: pair p+1's load/cast/
            # square/norm stages are emitted between pair p's two convs so
            # no engine queue blocks PSUM eviction for long
            NP = BLOC // 2
            xt2 = load_x(0)
            sqst = square_pair(0, xt2)
            xf8_a = prep_half(0, xt2, 0)
            xf8_b = prep_half(0, xt2, 1)
            sq_mms(sqst, range(NTILES))
            norm_pair(0, sqst)
            for p in range(NP):
                last = p + 1 >= NP
                if not last:
                    xt2_n = load_x(p + 1)
                conv_batch(2 * p, xf8_a, wdr, None, (0, 0))
                if not last:
                    sqst_n = square_pair(p + 1, xt2_n)
                    xf8_a = prep_half(p + 1, xt2_n, 0)
                conv_batch(
                    2 * p + 1, xf8_b, wdr_o,
                    None if last else sqst_n, (0, 8),
                )
                if not last:
                    xf8_b = prep_half(p + 1, xt2_n, 1)
                    norm_pair(p + 1, sqst_n)

            # ---- finish: out[b] = sum_f g[f]*S[f,b] + sum_f bias[f] ------
            pf = pconv.tile([1, BLOC], F32, name="pf", tag="pc")
            nc.tensor.matmul(pf[:], gsb[:], S[:])
            out_sb = constp.tile([1, BLOC], F32)
            nc.scalar.add(out_sb[:], pf[:], bsumsb[0:1, 0:1])
            nc.sync.dma_start(out_d[:], out_sb[:])

    ndup = dedupe_ldweights(nc)
    print(f"dedupe_ldweights: replaced {ndup}", file=sys.stderr)
    return nc


_PROGRAM: bass.Bass | None = None


def _get_program() -> bass.Bass:
    global _PROGRAM
    if _PROGRAM is None:
        _PROGRAM = build_program()
    return _PROGRAM


# ---------------------------------------------------------------------------
# Host entry point
# ---------------------------------------------------------------------------

def host_params(conv_weights, spat_weights, weight, bias):
    """Tiny host-side precomputation of stationaries and scalars."""
    conv = np.asarray(conv_weights, dtype=np.float64)
    spat = np.asarray(spat_weights, dtype=np.float64)
    w = np.asarray(weight, dtype=np.float64)
    bb = np.asarray(bias, dtype=np.float64)

    # prod[k, c, f] = conv[f, k] * spat[f, c]; DoubleRow packing
    # k = 32*j + 2*kq + k2  ->  wdr[(k2,c), kq, j, f]
    prod = np.einsum("fk,fc->kcf", conv, spat) * SW    # [K, C, F]
    P5 = prod.reshape(2, KQ, 2, CIN, F)                # [j, kq, k2, c, f]
    wdr = np.ascontiguousarray(P5.transpose(2, 3, 1, 0, 4)).reshape(
        128, KQ, 2, F
    )
    # SwInterleave layout: stored[p, kq, 2*(127-f) + j] = W[p, kq, j, f]
    wdr = np.ascontiguousarray(
        wdr[:, :, :, ::-1].transpose(0, 1, 3, 2)
    ).reshape(128, KQ, 2 * F)
    wdr = np.clip(wdr, -240.0, 240.0)
    f8np = mybir.dt.np(F8)
    wdr = wdr.astype(np.float32)
    wdro = np.roll(wdr, 64, axis=0)
    wdr = wdr.astype(f8np)
    wdro = wdro.astype(f8np)

    norm_w = np.sqrt((spat * spat).sum(1) * (conv * conv).sum(1))  # [F]
    g = (SCALE / (TOUT * norm_w * SW * SX) * w).astype(np.float32).reshape(F, 1)
    bsum = np.array([[bb.sum()]], dtype=np.float32)
    return wdr, wdro, g, bsum


def make_in_maps(x, conv_weights, spat_weights, weight, bias):
    x = np.ascontiguousarray(np.asarray(x, dtype=np.float32))
    wdr, wdro, g, bsum = host_params(conv_weights, spat_weights, weight, bias)
    bf16np = mybir.dt.np(BF16)
    ones2 = np.zeros((128, 2), np.float32)
    ones2[0:64, 0] = 1.0
    ones2[64:128, 1] = 1.0
    ones2 = ones2.astype(bf16np)
    in_maps = []
    for c in range(NCORES):
        in_maps.append(
            {
                "xs": np.ascontiguousarray(x[c * BLOC:(c + 1) * BLOC]),
                "wdr": wdr,
                "wdro": wdro,
                "g": g,
                "bsum": bsum,
                "ones2": ones2,
            }
        )
    return in_maps


def kernel(x, conv_weights, spat_weights, weight, bias):
    in_maps = make_in_maps(x, conv_weights, spat_weights, weight, bias)
    nc = _get_program()
    res = run_bass_kernel_spmd(nc, in_maps, core_ids=list(range(NCORES)))
    out = np.concatenate(
        [res.results[c]["out"].reshape(BLOC) for c in range(NCORES)]
    )
    return out.astype(np.float32)


# revision 34
# speedup vs baseline: 1.0003x; 1.0003x over previous
"""Trainium2 Bass kernel for nn_CosSimSpatTempConvNet.

Math (reference):
  merged[f,c,k] = conv_w[f,k] * spat_w[f,c]                  (rank-1 kernel)
  conved[b,f,t] = sum_{c,k} merged[f,c,k] * x[b,c,t+k]       (valid conv, Tout=T-K+1)
  norm_w[f]    = ||conv_w[f]|| * ||spat_w[f]||
  norm_in[b,t] = sqrt(sum_{c,k} x[b,c,t+k]^2)
  cos[b,f,t]   = conved * 64 / (norm_w[f] * norm_in[b,t])
  out[b]       = sum_f (mean_t |cos[b,f,t]| * weight[f] + bias[f])

Device strategy (8 cores, data-parallel over batch, 8 b per core):
  * Conv as fp8e4 DoubleRow TensorE matmuls: contraction 256 per
    instruction = 128 partitions (k2 in {0,1} x c) x 2 k-planes
    (j in {0,1}; k = 2*kq + k2 + 32*j).  16 PSUM-accumulated matmuls per
    512-wide time tile (vs 32 for fp32r), 0.5 cycles/output-row.
    Stationaries wdr[(k2,c), kq, j, f] = conv_w[f,2kq+k2+32j]*spat_w[f,c]
    host-precomputed, scaled by SW=128, clipped to +-240 (TRN e4m3).
  * Weights use the SwInterleave layout (host pre-interleaved pairs);
    matmuls run at ~97% of the fp8 TensorE roofline (~216ns per
    512-wide DR matmul incl. its LDWEIGHTS).
  * Moving operand xf8[(k2,c), j, t] = 16*x[c, t+k2+32j] in fp8, built
    by ScalarE/DVE casts + one SBUF shift-DMA.  Batches are processed
    in PAIRS ([128, T] x-load, one Square, [2, 512] c-sum matmuls);
    the odd batch keeps its planes on partitions 64-127 and uses the
    row-rolled stationary wdr_o, so no cross-partition engine ops.
  * norm_in: x^2 (ScalarE), sum over c via ones-stationary matmul,
    then per-pair chunked layout [32 chunks, 256+64 halo] so the
    sliding-window-64 doubling shifts + reciprocal run on short rows.
  * epilogue per (b, t-tile): 1/norm row staged to DRAM and DMA-
    broadcast across 128 partitions (partition_broadcast AP, no PE);
    |conved| via ScalarE Abs; fused multiply+row-sum on DVE
    (accum_out), all in bf16.
  * software pipeline: pair p+1's load/casts/squares/norms are emitted
    between pair p's convs; next-pair c-sum matmuls interleave between
    conv waves to keep the PE stream dense.  A post-pass rewrites
    back-to-back duplicate InstLdweights into NOPs.
  * finish: S[f,b] sums -> one matmul with stationary
    g[f] = 64*weight/(4033*norm_w*SW*SX) contracting over f, + sum(bias).
"""

import contextlib
import ctypes
import sys
import types

import numpy as np

import concourse.bass as bass
import concourse.mybir as mybir
import concourse.tile as tile
from concourse.bass_utils import run_bass_kernel_spmd
from concourse.vector_clock import ScopedClock

F32 = mybir.dt.float32
F32R = mybir.dt.float32r
BF16 = mybir.dt.bfloat16
F8 = mybir.dt.float8e4

B, CIN, T = 64, 64, 4096
F, K = 128, 64
TOUT = T - K + 1          # 4033
NCORES = 8
BLOC = B // NCORES        # 8 batches per core
KQ = 16                   # DoubleRow quad groups: k = 2*kq + k2 + 32*j
TS = 512                  # moving-operand tile (one fp32 PSUM bank)
NTILES = (TOUT + TS - 1) // TS      # 8 (last tile 449)
SCALE = 64.0              # sqrt(CIN*K)
SW = 128.0                # fp8 weight scale
SX = 16.0                 # fp8 x scale
NCH = 16                  # norm chunks per batch
CW = 256                  # chunk output width
CPAD = 320                # chunk width incl. 64-halo
WAVE = 3                  # conv tiles accumulating concurrently (PSUM banks)

AF = mybir.ActivationFunctionType
ALU = mybir.AluOpType
DR = mybir.MatmulPerfMode.DoubleRowSwInterleave


# ---------------------------------------------------------------------------
# Container fixups: walrus here rejects >1 sem-wait on a Drain; TileContext's
# tail drain carries one wait per logical processor.  Chunk into single-wait
# drains.  Also recreate the (absent) antenv.axon_hooks NTFF profile hook so
# trace=True works when a test harness wants timings.
# ---------------------------------------------------------------------------

def _patched_drain_and_barrier(self, tick_clock, wait_clock):
    nc = self.nc
    drain_inst = nc.sync.drain()
    wait_clock.add_sem_waits(
        drain_inst.ins, ScopedClock({None: tick_clock.global_clock})
    )
    si = drain_inst.ins.sync_info
    waits = list(si.on_wait or []) if si else []
    if len(waits) > 1:
        si.on_wait = waits[:1]
        for w in waits[1:]:
            d2 = nc.sync.drain()
            si2 = d2.ins.sync_info
            if si2 is None:
                d2.ins.sync_info = mybir.SyncInfo(on_wait=[w], on_update=[])
            else:
                si2.on_wait = [w]
    nc.all_engine_barrier()
    assert self.sems is not None
    popped = nc._tile_sem_poison_stack.pop()
    assert popped is self._sem_poison
    nc.clear_and_free_semaphores(list(self.sems.allocated().values()))
    nc.all_engine_barrier()


def _install_ntff_hook():
    if "antenv.axon_hooks" in sys.modules:
        return
    try:
        lib = ctypes.CDLL("/opt/axon/libaxon_pjrt.so")
    except OSError:
        return
    if not hasattr(lib, "axon_start_nrt_profile"):
        return
    lib.axon_start_nrt_profile.argtypes = [
        ctypes.POINTER(ctypes.c_int64),
        ctypes.c_size_t,
    ]
    lib.axon_start_nrt_profile.restype = ctypes.c_int64
    lib.axon_stop_nrt_profile.argtypes = [ctypes.c_char_p]
    lib.axon_stop_nrt_profile.restype = ctypes.c_int64

    @contextlib.contextmanager
    def _hook(output_dir, device_ids):
        import jax

        jax.devices()
        if device_ids:
            ids = (ctypes.c_int64 * len(device_ids))(*device_ids)
            rc = lib.axon_start_nrt_profile(ids, len(device_ids))
        else:
            rc = lib.axon_start_nrt_profile(None, 0)
        if rc != 0:
            raise RuntimeError(f"axon_start_nrt_profile rc={rc}")
        try:
            yield
        finally:
            n = lib.axon_stop_nrt_profile(str(output_dir).encode())
            print(f"profile: {n} ntff file(s) in {output_dir}", file=sys.stderr)

    mod = types.ModuleType("antenv.axon_hooks")
    mod.get_axon_ntff_profile_hook = lambda: _hook
    mod.set_axon_ntff_profile_hook = lambda h: None
    import antenv

    antenv.axon_hooks = mod
    sys.modules["antenv.axon_hooks"] = mod


_ORIG_COMMIT = tile.TileContext._commit_instruction


def _commit_split_waits(self, inst, lazy_reg_writes=True):
    """walrus here allows only one sem-wait per instruction; move extras
    onto same-engine NOPs committed immediately before the instruction."""
    si = getattr(inst, "sync_info", None)
    if (
        si is not None
        and si.on_wait
        and len(si.on_wait) > 1
        and inst.engine != mybir.EngineType.Unassigned
    ):
        waits = list(si.on_wait)
        si.on_wait = waits[:1]
        for i, w in enumerate(waits[1:]):
            nop = mybir.InstNoOp(
                name=f"{inst.name}-wsplit{i}", ins=[], outs=[]
            )
            nop.engine = inst.engine
            nop.sync_info = mybir.SyncInfo(on_wait=[w], on_update=[])
            _ORIG_COMMIT(self, nop, lazy_reg_writes=False)
    return _ORIG_COMMIT(self, inst, lazy_reg_writes)


def install_fixups():
    tile.TileContext._drain_and_barrier = _patched_drain_and_barrier
    tile.TileContext._commit_instruction = _commit_split_waits
    _install_ntff_hook()


def dedupe_ldweights(nc: bass.Bass) -> int:
    """Replace back-to-back InstLdweights that reload the identical
    stationary with same-sync NOPs.  walrus emits one LDWEIGHTS per
    non-f32 matmul; in a kq-outer wave the followers reload the same
    weights, costing ~120ns of PE each for nothing.  Weights persist in
    the array across matmuls, so a NOP carrying the original sync_info
    is semantically equivalent."""
    n = 0
    for fn in nc.m.functions:
        for bb in fn.blocks:
            last_key = None
            insts = bb.instructions
            for i, inst in enumerate(insts):
                if isinstance(inst, mybir.InstLdweights):
                    key = repr(inst.ins)
                    if key == last_key:
                        nop = mybir.InstNoOp(
                            name=f"{inst.name}-wdup", ins=[], outs=[]
                        )
                        nop.engine = inst.engine
                        nop.sync_info = inst.sync_info
                        insts[i] = nop
                        n += 1
                    else:
                        last_key = key
                elif isinstance(inst, mybir.InstMatmult):
                    if inst.ldweights is not False:
                        last_key = None
                elif getattr(inst, "engine", None) == getattr(
                    mybir.EngineType, "PE", None
                ):
                    last_key = None
    return n


# ---------------------------------------------------------------------------
# Device program (identical on all 8 cores; inputs differ per core)
# ---------------------------------------------------------------------------

def build_program() -> bass.Bass:
    install_fixups()
    nc = bass.Bass()

    xs_in = nc.dram_tensor("xs", [BLOC, CIN, T], F32, kind="ExternalInput")
    wdr_in = nc.dram_tensor("wdr", [128, KQ, 2 * F], F8, kind="ExternalInput")
    wdro_in = nc.dram_tensor("wdro", [128, KQ, 2 * F], F8, kind="ExternalInput")
    g_in = nc.dram_tensor("g", [F, 1], F32, kind="ExternalInput")
    bsum_in = nc.dram_tensor("bsum", [1, 1], F32, kind="ExternalInput")
    ones2_in = nc.dram_tensor("ones2", [128, 2], BF16, kind="ExternalInput")
    out_d = nc.dram_tensor("out", [1, BLOC], F32, kind="ExternalOutput")
    rdram = nc.dram_tensor("rdram", [BLOC, NCH, CW], BF16, kind="Internal")

    PS = bass.MemorySpace.PSUM

    with tile.TileContext(nc) as tc:
        with (
            tc.tile_pool(name="const", bufs=1) as constp,
            tc.tile_pool(name="xtp", bufs=2) as xtp,
            tc.tile_pool(name="sqp", bufs=2) as sqp,
            tc.tile_pool(name="xf8p", bufs=4) as xf8p,
            tc.tile_pool(name="rowp", bufs=2) as rowp,
            tc.tile_pool(name="slidep", bufs=3) as slidep,
            tc.tile_pool(name="rcp", bufs=2) as rcp,
            tc.tile_pool(name="pbsp", bufs=3) as pbsp,
            tc.tile_pool(name="scrp", bufs=3) as scrp,
            tc.tile_pool(name="accp", bufs=2) as accp,
            tc.tile_pool(name="pconv", bufs=2 * WAVE, space=PS) as pconv,
            tc.tile_pool(name="psq", bufs=2, space=PS) as psq,
        ):
            # constants
            wdr = constp.tile([128, KQ, 2 * F], F8)
            nc.sync.dma_start(wdr[:], wdr_in[:])
            wdr_o = constp.tile([128, KQ, 2 * F], F8)
            nc.sync.dma_start(wdr_o[:], wdro_in[:])
            gsb = constp.tile([F, 1], F32)
            nc.sync.dma_start(gsb[:], g_in[:])
            bsumsb = constp.tile([1, 1], F32)
            nc.sync.dma_start(bsumsb[:], bsum_in[:])
            ones2 = constp.tile([128, 2], BF16)
            nc.sync.dma_start(ones2[:], ones2_in[:])

            S = constp.tile([F, BLOC], F32)        # per-(f,b) |cos| sums

            def load_x(p):
                """DMA x for batches (2p, 2p+1)."""
                xt2 = xtp.tile([128, T], F32, tag="xt", name=f"xt{p}")
                nc.sync.dma_start(xt2[:], xs_in[2 * p:2 * p + 2])
                return xt2

            def square_pair(p, xt2):
                """One Square for the pair + norm staging row."""
                xsq2 = sqp.tile([128, T], BF16, tag="xsq", name=f"xsq{p}")
                nc.scalar.activation(xsq2[:], xt2[:], AF.Square)
                srow2 = rowp.tile(
                    [2, NCH + 1, CW], BF16, tag="srow", name=f"srow{p}"
                )
                nc.vector.memset(srow2[0:2, NCH, :], 1.0)
                return {"xsq": xsq2, "srow": srow2}

            def prep_half(p, xt2, h):
                """fp8 moving operand for batch 2p+h via ScalarE/DVE casts.
                The odd batch writes its (k2=0) planes on partitions 64-127
                (same lanes as its x rows); conv then uses the row-rolled
                stationary wdr_o."""
                xf8 = xf8p.tile(
                    [128, 2, T], F8, tag="xf8", name=f"xf8_{2 * p + h}"
                )
                lo, hi = (0, 64) if h == 0 else (64, 128)
                src = xt2[lo:hi, :]
                nc.vector.memset(xf8[:, 0:2, T - 64:T], 0.0)
                nc.scalar.activation(
                    xf8[lo:hi, 0, 0:T], src[:, 0:T], AF.Copy, scale=SX
                )
                nc.vector.tensor_scalar_mul(
                    xf8[lo:hi, 1, 0:T - 32], src[:, 32:T], SX
                )
                # shifted (k2=1) planes land on the other 64 partitions
                olo = 64 - lo
                nc.sync.dma_start(
                    xf8[olo:olo + 64, 0:2, 0:T - 1],
                    xf8[lo:hi, 0:2, 1:T],
                )
                return xf8

            def sq_mms(st, tslist):
                """Pair c-sum matmuls (interleavable between conv waves):
                ones2 columns pick upper/lower 64 partitions -> [2, TS]."""
                for ts in tslist:
                    pq = psq.tile([2, TS], F32, name="pq", tag="pq")
                    nc.tensor.matmul(
                        pq[:],
                        ones2[:],
                        st["xsq"][:, ts * TS:(ts + 1) * TS],
                    )
                    nc.scalar.copy(
                        st["srow"][0:2, 2 * ts:2 * ts + 2, :], pq[:]
                    )

            def norm_pair(p, st):
                """Sliding-window-64 sums + 1/sqrt for a batch pair;
                results to rdram[2p], rdram[2p+1]."""
                b0 = 2 * p
                srow2 = st["srow"]
                sqc = slidep.tile([2 * NCH, CPAD], BF16, tag="slide",
                                  name=f"sqc{p}")
                for h in range(2):
                    r0 = NCH * h
                    nc.sync.dma_start(
                        sqc[r0:r0 + NCH, 0:CW],
                        srow2[h:h + 1, 0:NCH, :],
                    )
                    nc.sync.dma_start(
                        sqc[r0:r0 + NCH, CW:CPAD],
                        srow2[h:h + 1, 1:NCH + 1, 0:64],
                    )
                cur = sqc
                width = CPAD
                for sh in (1, 2, 4, 8, 16, 32):
                    width -= sh
                    nxt = slidep.tile([2 * NCH, CPAD], BF16, tag="slide",
                                      name=f"sl{p}_{sh}")
                    nc.vector.tensor_tensor(
                        nxt[:, 0:width], cur[:, 0:width],
                        cur[:, sh:sh + width], op=ALU.add,
                    )
                    cur = nxt
                assert width == CW + 1
                rec = rowp.tile([2 * NCH, CPAD], F32, tag="rec",
                                name=f"rec{p}")
                nc.vector.reciprocal(rec[:, 0:CW], cur[:, 0:CW])
                rc = rcp.tile([2 * NCH, CW], BF16, tag="rc", name=f"rc{p}")
                nc.scalar.activation(rc[:], rec[:, 0:CW], AF.Sqrt)
                nc.sync.dma_start(rdram[b0], rc[0:NCH, :])
                nc.sync.dma_start(rdram[b0 + 1], rc[NCH:2 * NCH, :])

            def conv_batch(b, xf8, wtile, inter_st, sq_span):
                """Conv waves + cosine epilogue; sq matmuls of the *next*
                pair (inter_st) are interleaved between waves to keep the
                PE stream dense."""
                lo, hi = sq_span
                acc = accp.tile([F, NTILES], F32, name=f"acc{b}")
                sq_chunks = [(0, 3), (3, 6), (6, 8)]
                for wi, w0 in enumerate(range(0, NTILES, WAVE)):
                    wtiles = list(range(w0, min(w0 + WAVE, NTILES)))
                    pcs = {}
                    for ts in wtiles:
                        pcs[ts] = pconv.tile(
                            [F, TS], F32, name=f"pc_{b}_{ts}", tag="pc"
                        )
                    # kq-outer: one stationary serves len(wtiles) matmuls;
                    # followers skip the redundant LDWEIGHTS
                    for kq in range(KQ):
                        for i, ts in enumerate(wtiles):
                            t0 = ts * TS
                            nt = min(TS, TOUT - t0)
                            nt_mm = nt + (nt & 1)
                            bi = nc.tensor.matmul(
                                pcs[ts][:, 0:nt_mm],
                                wtile[:, kq, :],
                                xf8[:, 0:2, t0 + 2 * kq: t0 + 2 * kq + nt_mm],
                                start=(kq == 0),
                                stop=(kq == KQ - 1),
                                perf_mode=DR,
                            )
                            if i > 0:
                                bi.ins.ldweights = False
                    if inter_st is not None:
                        lo, hi = sq_chunks[wi]
                        sq_mms(inter_st, range(lo, hi))
                    for ts in wtiles:
                        t0 = ts * TS
                        nt = min(TS, TOUT - t0)
                        # 1/norm row, broadcast across partitions by DMA
                        # from the DRAM copy (not PE)
                        pbs = pbsp.tile(
                            [128, TS], BF16, tag="pbs", name=f"pbs{b}_{ts}"
                        )
                        nc.sync.dma_start(
                            pbs[:],
                            rdram[b, 2 * ts:2 * ts + 2, 0:CW]
                            .partition_broadcast(128),
                        )
                        # DVE may read only one PSUM operand: take |conv| on
                        # ScalarE (PSUM->SBUF), then multiply+row-sum on DVE.
                        scr = scrp.tile(
                            [F, TS], BF16, tag="scr", name=f"scr{b}_{ts}"
                        )
                        nc.scalar.activation(
                            scr[:, 0:nt], pcs[ts][:, 0:nt], AF.Abs
                        )
                        nc.vector.scalar_tensor_tensor(
                            scr[:, 0:nt],
                            scr[:, 0:nt],
                            1.0,
                            pbs[:, 0:nt],
                            op0=ALU.mult,
                            op1=ALU.mult,
                            accum_out=acc[:, ts:ts + 1],
                        )
                nc.vector.reduce_sum(
                    S[:, b:b + 1], acc[:], axis=mybir.AxisListType.X
                )

            # software pipeline over batch pairs: pair p+1's load/cast/
            # square/norm stages are emitted between pair p's two convs so
            # no engine queue blocks PSUM eviction for long
            NP = BLOC // 2
            xt2 = load_x(0)
            sqst = square_pair(0, xt2)
            xf8_a = prep_half(0, xt2, 0)
            xf8_b = prep_half(0, xt2, 1)
            sq_mms(sqst, range(NTILES))
            norm_pair(0, sqst)
            for p in range(NP):
                last = p + 1 >= NP
                if not last:
                    xt2_n = load_x(p + 1)
                conv_batch(2 * p, xf8_a, wdr, None, (0, 0))
                if not last:
                    sqst_n = square_pair(p + 1, xt2_n)
                    xf8_a = prep_half(p + 1, xt2_n, 0)
                conv_batch(
                    2 * p + 1, xf8_b, wdr_o,
                    None if last else sqst_n, (0, 8),
                )
                if not last:
                    xf8_b = prep_half(p + 1, xt2_n, 1)
                    norm_pair(p + 1, sqst_n)

            # ---- finish: out[b] = sum_f g[f]*S[f,b] + sum_f bias[f] ------
            pf = pconv.tile([1, BLOC], F32, name="pf", tag="pc")
            nc.tensor.matmul(pf[:], gsb[:], S[:])
            out_sb = constp.tile([1, BLOC], F32)
            nc.scalar.add(out_sb[:], pf[:], bsumsb[0:1, 0:1])
            nc.sync.dma_start(out_d[:], out_sb[:])

    ndup = dedupe_ldweights(nc)
    print(f"dedupe_ldweights: replaced {ndup}", file=sys.stderr)
    return nc


_PROGRAM: bass.Bass | None = None


def _get_program() -> bass.Bass:
    global _PROGRAM
    if _PROGRAM is None:
        _PROGRAM = build_program()
    return _PROGRAM


# ---------------------------------------------------------------------------
# Host entry point
# ---------------------------------------------------------------------------

def host_params(conv_weights, spat_weights, weight, bias):
    """Tiny host-side precomputation of stationaries and scalars."""
    conv = np.asarray(conv_weights, dtype=np.float64)
    spat = np.asarray(spat_weights, dtype=np.float64)
    w = np.asarray(weight, dtype=np.float64)
    bb = np.asarray(bias, dtype=np.float64)

    # prod[k, c, f] = conv[f, k] * spat[f, c]; DoubleRow packing
    # k = 32*j + 2*kq + k2  ->  wdr[(k2,c), kq, j, f]
    prod = np.einsum("fk,fc->kcf", conv, spat) * SW    # [K, C, F]
    P5 = prod.reshape(2, KQ, 2, CIN, F)                # [j, kq, k2, c, f]
    wdr = np.ascontiguousarray(P5.transpose(2, 3, 1, 0, 4)).reshape(
        128, KQ, 2, F
    )
    # SwInterleave layout: stored[p, kq, 2*(127-f) + j] = W[p, kq, j, f]
    wdr = np.ascontiguousarray(
        wdr[:, :, :, ::-1].transpose(0, 1, 3, 2)
    ).reshape(128, KQ, 2 * F)
    wdr = np.clip(wdr, -240.0, 240.0)
    f8np = mybir.dt.np(F8)
    wdr = wdr.astype(np.float32)
    wdro = np.roll(wdr, 64, axis=0)
    wdr = wdr.astype(f8np)
    wdro = wdro.astype(f8np)

    norm_w = np.sqrt((spat * spat).sum(1) * (conv * conv).sum(1))  # [F]
    g = (SCALE / (TOUT * norm_w * SW * SX) * w).astype(np.float32).reshape(F, 1)
    bsum = np.array([[bb.sum()]], dtype=np.float32)
    return wdr, wdro, g, bsum


def make_in_maps(x, conv_weights, spat_weights, weight, bias):
    x = np.ascontiguousarray(np.asarray(x, dtype=np.float32))
    wdr, wdro, g, bsum = host_params(conv_weights, spat_weights, weight, bias)
    bf16np = mybir.dt.np(BF16)
    ones2 = np.zeros((128, 2), np.float32)
    ones2[0:64, 0] = 1.0
    ones2[64:128, 1] = 1.0
    ones2 = ones2.astype(bf16np)
    in_maps = []
    for c in range(NCORES):
        in_maps.append(
            {
                "xs": np.ascontiguousarray(x[c * BLOC:(c + 1) * BLOC]),
                "wdr": wdr,
                "wdro": wdro,
                "g": g,
                "bsum": bsum,
                "ones2": ones2,
            }
        )
    return in_maps


def kernel(x, conv_weights, spat_weights, weight, bias):
    in_maps = make_in_maps(x, conv_weights, spat_weights, weight, bias)
    nc = _get_program()
    res = run_bass_kernel_spmd(nc, in_maps, core_ids=list(range(NCORES)))
    out = np.concatenate(
        [res.results[c]["out"].reshape(BLOC) for c in range(NCORES)]
    )
    return out.astype(np.float32)
